# revision 39
# baseline (speedup 1.0000x reference)
"""Trainium2 Bass kernel for GQA causal attention (B=2, S=2048, D=2048,
16 q-heads / 4 kv-heads, head_dim=128, interleaved RoPE).

Sharding: DP=2 over batch x TP=4 over head groups (8 cores).
Core c: batch b=c//4, rank r=c%4 -> q-heads [4r,4r+4), kv-head r.
Each core computes its heads' attention output (transposed layout [e,s]);
after each 512-col chunk j an AllToAll reshards that chunk's columns to
their owner cores (core c owns cols j*512 + c*64 of both batches); the
output projection for all four chunks runs after the last trigger so the
final AllToAll hides under chunk 0-2 out-proj work.

v7: single merged projection/attention pipeline (trace-driven).
 - chunk j+1's projection m-tiles (k, v, q0..q3, all pure-PE work) are
   interleaved between chunk j's attention groups, so every exp ACT /
   mask / softmax-finalize chain has PE cover and the phase-1 -> 2
   boundary stalls disappear; attention chunk 0 likewise covers the
   xT chunk-1 DMA.  PSUM rebalanced for this: two 2-bank score pools
   (uniform 2-tile groups), 2x1-bank AV accumulators, and a 2x1-bank
   shared pool for proj psum / rope swap / v-transpose / den sums.
 - every bulk load is ONE multi-block DMA (a single InstDMACopy is
   split across all 16 SDMA engines): per-128KB-instruction issue cost
   capped the stream at ~200GB/s and starved the PE at start.
 - attention tail (mask+den+AV) lags the score/exp pipeline by TWO
   groups (~2us of PE cover per exp ACT).
 - one AllToAll per chunk (0.5MB); a2a block rows are (e, local_head)
   so the phase-3 import gathers 512B runs (one DMA per chunk) into a
   raw tile, then a DVE column-permute makes each head's lhsT a
   contiguous [128, 128] slice.  Imports issue on the scalar DGE ring
   pinned past the end of the model schedule: on the sync ring their
   collective-wait head-of-line-blocks the exports / out-writes queued
   behind them.
"""

import math
import sys

sys.path.insert(0, "/opt/trn_rl_repo")

from contextlib import ExitStack

import ml_dtypes
import numpy as np

import concourse.bass as bass
import concourse.mybir as mybir
import concourse.tile as tile
from concourse import bacc
from concourse.bass_utils import run_bass_kernel_spmd
from concourse.masks import make_identity

BF16 = mybir.dt.bfloat16
F16 = mybir.dt.float16
F32 = mybir.dt.float32

N_HEADS = 16
N_KV_HEADS = 4
HD = 128
ROPE_THETA = 10000.0
TP = 4
N_CORES = 8


def build_graph(S=2048, D=2048, HQL=4, NS=512):
    """Per-core SPMD graph. HQL = local q heads; local kv heads = 1."""
    hd = HD
    ND = D // 128          # d-tiles (projection contraction tiles)
    NC = S // NS           # s-chunks == AllToAll count
    MQ = HQL * hd          # local q width
    DIAG = NS // 128       # sk-tiles per chunk needing a causal mask
    NB = N_CORES // TP     # batches
    SW = NS // N_CORES     # strip width per (chunk, dst core)
    NSW = NS // SW         # dst blocks per export (= N_CORES)
    OW = NC * SW           # out cols per core per batch
    P2 = NB * SW           # fused out-proj psum partition rows
    scale = 1.0 / math.sqrt(hd)
    NH = TP * HQL          # global head count

    nc = bacc.Bacc("TRN2", target_bir_lowering=False, debug=False,
                   num_devices=N_CORES)

    xT_e = nc.dram_tensor("xT", [D, S], BF16, kind="ExternalInput").ap()
    wqT_e = nc.dram_tensor("wqT", [D, MQ], BF16, kind="ExternalInput").ap()
    wkT_e = nc.dram_tensor("wkT", [D, hd], BF16, kind="ExternalInput").ap()
    wvT_e = nc.dram_tensor("wvT", [D, hd], BF16, kind="ExternalInput").ap()
    woT_e = nc.dram_tensor("woT", [NH * hd, D], BF16,
                           kind="ExternalInput").ap()
    ccx_e = nc.dram_tensor("ccx", [128, S], BF16, kind="ExternalInput").ap()
    ssx_e = nc.dram_tensor("ssx", [128, S], BF16, kind="ExternalInput").ap()
    psw_e = nc.dram_tensor("pswap", [128, 128], BF16,
                           kind="ExternalInput").ap()
    mask_e = nc.dram_tensor("mask", [128, NS + 384], F16,
                            kind="ExternalInput").ap()
    out_e = nc.dram_tensor("out", [NB * OW, D], F32,
                           kind="ExternalOutput").ap()

    a2a_in = [nc.dram_tensor(f"a2a_in{m}", [N_CORES * MQ, SW], BF16)
              for m in range(NC)]
    a2a_out = [nc.dram_tensor(f"a2a_out{m}", [N_CORES * MQ, SW], BF16)
               for m in range(NC)]
    groups = [list(range(N_CORES))]

    with tile.TileContext(nc) as tc, ExitStack() as ctx:
        ep = ctx.enter_context
        const_pool = ep(tc.tile_pool(name="const", bufs=1))
        rt_pool = ep(tc.tile_pool(name="rt", bufs=HQL + 1))
        vst_pool = ep(tc.tile_pool(name="vst", bufs=1))
        ptw_pool = ep(tc.tile_pool(name="ptw", bufs=3))
        den_pool = ep(tc.tile_pool(name="den", bufs=2))
        rc_pool = ep(tc.tile_pool(name="rc", bufs=1))
        rbc_pool = ep(tc.tile_pool(name="rbc", bufs=2))
        attn_pool = ep(tc.tile_pool(name="attn", bufs=3))
        osb_pool = ep(tc.tile_pool(name="osb", bufs=2))
        wo_pool = ep(tc.tile_pool(name="wo", bufs=1))
        aot_pool = ep(tc.tile_pool(name="aot", bufs=2))
        aotr_pool = ep(tc.tile_pool(name="aotr", bufs=1))
        xt_pool = ep(tc.tile_pool(name="xt", bufs=2))
        wq_pool = ep(tc.tile_pool(name="wqp", bufs=1))
        wkv_pool = ep(tc.tile_pool(name="wkv", bufs=1))
        stage_pool = ep(tc.tile_pool(name="stg", bufs=3))
        tmp_pool = ep(tc.tile_pool(name="tmp", bufs=4))
        # PSUM: 8 banks, statically partitioned:
        #   scwA 1x2 + scwB 1x2 + atp 2x1 + pjd 2x1 = 8
        # pjd is shared by proj psum / rope-swap / v-transpose / den-sum
        scwA_pool = ep(tc.tile_pool(name="scwA", bufs=1, space="PSUM"))
        scwB_pool = ep(tc.tile_pool(name="scwB", bufs=1, space="PSUM"))
        atp_pool = ep(tc.tile_pool(name="atp", bufs=2, space="PSUM"))
        pjd_pool = ep(tc.tile_pool(name="pjd", bufs=2, space="PSUM"))

        # ---- constants ----
        ident = const_pool.tile([128, 128], BF16, tag="ident")
        make_identity(nc, ident[:])
        ones = const_pool.tile([128, 1], F16, tag="ones")
        nc.gpsimd.memset(ones[:], 1.0)
        warm = const_pool.tile([1, 8], F32, tag="warm")
        nc.gpsimd.memset(warm[:], 0.0)
        # preload the exp ACT table set before attention needs it
        nc.scalar.activation(warm[:], warm[:],
                             mybir.ActivationFunctionType.Exp)
        ccx = const_pool.tile([128, S], BF16, tag="ccx")
        ssx = const_pool.tile([128, S], BF16, tag="ssx")
        msk = const_pool.tile([128, NS + 384], F16, tag="msk")

        vst = vst_pool.tile([128, S], BF16, tag="vst")   # vT staging
        vnat = vst_pool.tile([128, S], BF16, tag="vnat")  # v [sk, e] blocks

        # ---- DMA stream (priority order; each a single multi-block
        # DMA so it spreads across all 16 SDMA engines) ----
        wk_all = wkv_pool.tile([128, ND * hd], BF16, tag="wk")
        nc.sync.dma_start(
            wk_all[:].rearrange("p (d h) -> p d h", d=ND),
            wkT_e.rearrange("(d p) h -> p d h", p=128))
        xts = []
        for sp in range(NC):
            xts.append(xt_pool.tile([128, ND * NS], BF16, tag="xt",
                                    name=f"xt{sp}"))
        # chunk 0 in pieces (smallest first) so the k-projection's
        # leading d-tiles can start as early as possible
        if ND >= 8:
            pieces = [2, 2] + [4] * ((ND - 4) // 4)
        else:
            pieces = [2] * (ND // 2)
        q0 = 0
        for pc in pieces:
            nc.sync.dma_start(
                xts[0][:, q0 * NS:(q0 + pc) * NS].rearrange(
                    "p (d s) -> p d s", d=pc),
                xT_e.rearrange("(d p) s -> p d s",
                               p=128)[:, q0:q0 + pc, 0:NS])
            q0 += pc
        wv_all = wkv_pool.tile([128, ND * hd], BF16, tag="wv")
        nc.sync.dma_start(
            wv_all[:].rearrange("p (d h) -> p d h", d=ND),
            wvT_e.rearrange("(d p) h -> p d h", p=128))
        # rope tables: chunk-0 columns first, the rest after xT chunk 1
        nc.sync.dma_start(ccx[:, 0:NS], ccx_e[:, 0:NS])
        nc.sync.dma_start(ssx[:, 0:NS], ssx_e[:, 0:NS])
        # wq per head, in consumption order, so xT chunk 1 isn't stuck
        # behind the full 2MB
        wq_all = wq_pool.tile([128, ND * MQ], BF16, tag="wq")
        for h in range(HQL):
            nc.sync.dma_start(
                wq_all[:].rearrange("p (d m) -> p d m",
                                    d=ND)[:, :, h * hd:(h + 1) * hd],
                wqT_e.rearrange("(d p) m -> p d m",
                                p=128)[:, :, h * hd:(h + 1) * hd])
        nc.sync.dma_start(
            xts[1][:].rearrange("p (d s) -> p d s", d=ND),
            xT_e.rearrange("(d p) s -> p d s", p=128)[:, :, NS:2 * NS])
        nc.sync.dma_start(
            ccx[:, NS:].rearrange("p (b c) -> p b c", b=NC - 1),
            ccx_e[:, NS:].rearrange("p (b c) -> p b c", b=NC - 1))
        nc.sync.dma_start(
            ssx[:, NS:].rearrange("p (b c) -> p b c", b=NC - 1),
            ssx_e[:, NS:].rearrange("p (b c) -> p b c", b=NC - 1))
        nc.sync.dma_start(msk[:], mask_e[:])
        for sp in range(2, NC):
            nc.sync.dma_start(
                xts[sp][:].rearrange("p (d s) -> p d s", d=ND),
                xT_e.rearrange("(d p) s -> p d s",
                               p=128)[:, :, sp * NS:(sp + 1) * NS])
        wo_all = wo_pool.tile([128, NH * D], BF16, tag="wo")
        nc.sync.dma_start(
            wo_all[:].rearrange("p (t c) -> p t c", t=NH),
            woT_e.rearrange("(t p) c -> p t c", p=128))

        def wo_t(ht):
            return wo_all[:, ht * D:(ht + 1) * D]

        # ---- projections + rope (emitted as per-chunk unit lists,
        # interleaved into the previous chunk's attention below) ----
        # Rope is software-pipelined one m-tile behind the projection
        # matmuls: the swap-MM of item k is emitted after the proj
        # group of item k+1 so it never stalls the PE waiting for the
        # ACT psum->stg copy.
        rope_pend = []   # queue of (stg, swap-psum, rt_tile, ssl)

        def rope_tail():
            if not rope_pend:
                return
            stg, ps2, rt_tile, ssl = rope_pend.pop(0)
            t1 = tmp_pool.tile([128, NS], BF16, tag="tmp")
            nc.vector.tensor_mul(t1[:], stg[:], ccx[:, ssl])
            t2 = tmp_pool.tile([128, NS], BF16, tag="tmp")
            nc.vector.tensor_mul(t2[:], ps2[:], ssx[:, ssl])
            nc.vector.tensor_add(rt_tile[:, ssl], t1[:], t2[:])

        def rope_swap_mm():
            # partition-half swap via two SBUF->SBUF DMAs instead of a
            # PE permutation matmul: frees ~5us of PE (the bottleneck
            # engine) and the DVE tail then reads bf16 SBUF (2x mode)
            # instead of f32 PSUM (1x).  Latency hides behind the one
            # m-tile of pipeline lag.
            if not rope_pend:
                return
            stg, _, rt_tile, ssl = rope_pend[0]
            sw = tmp_pool.tile([128, NS], BF16, tag="tmp",
                               name="rswap")
            nc.sync.dma_start(sw[0:64, :], stg[64:128, :])
            nc.sync.dma_start(sw[64:128, :], stg[0:64, :])
            rope_pend[0] = (stg, sw, rt_tile, ssl)

        def proj_one(lhs_all, lw, mslice, is_v, rt_tile, sp):
            ssl = slice(sp * NS, (sp + 1) * NS)
            ps = pjd_pool.tile([128, NS], F32, tag="pjd", name="psp")
            for d in range(ND):
                nc.tensor.matmul(
                    ps[:],
                    lhs_all[:, d * lw + mslice.start:
                            d * lw + mslice.stop],
                    xts[sp][:, d * NS:(d + 1) * NS],
                    start=(d == 0), stop=(d == ND - 1))
            rope_swap_mm()
            rope_tail()
            if is_v:
                nc.scalar.copy(vst[:, ssl], ps[:])
            else:
                stg = stage_pool.tile([128, NS], BF16, tag="stg")
                nc.scalar.copy(stg[:], ps[:])
                rope_pend.append((stg, None, rt_tile, ssl))

        def v_transpose(sp):
            for lt in range(DIAG):
                st = sp * DIAG + lt
                tpp = pjd_pool.tile([128, 128], BF16, tag="pjd",
                                    name="pst")
                nc.tensor.transpose(
                    tpp[:], vst[:, st * 128:(st + 1) * 128], ident[:])
                nc.scalar.copy(vnat[:, st * 128:(st + 1) * 128], tpp[:])

        def rope_flush():
            rope_swap_mm()
            rope_tail()
            rope_swap_mm()
            rope_tail()

        krt = rt_pool.tile([128, S], BF16, tag="rt", name="rtk")
        rts = [rt_pool.tile([128, S], BF16, tag="rt", name=f"rtq{h}")
               for h in range(HQL)]

        def proj_units(sp):
            """Pure-PE unit closures projecting chunk sp."""
            us = [lambda sp=sp: proj_one(wk_all, hd, slice(0, hd),
                                         False, krt, sp),
                  lambda sp=sp: proj_one(wv_all, hd, slice(0, hd),
                                         True, None, sp),
                  lambda sp=sp: v_transpose(sp)]
            for h in range(HQL):
                us.append(lambda sp=sp, h=h: proj_one(
                    wq_all, MQ, slice(h * hd, (h + 1) * hd),
                    False, rts[h], sp))
            us.append(rope_flush)
            return us

        # ---- attention machinery ----
        # One head per pass; sk-tiles processed in uniform 2-tile
        # groups alternating between the two 2-bank score pools, ONE
        # exp ACTIVATE per group.  The mask/den/AV tail lags the
        # score+exp pipeline by TWO groups; a pass's finalize (den
        # fold+reduce, recip, broadcast) and export (normalize + DMA +
        # AllToAll) are deferred into the following groups.
        class Pass:
            def __init__(self, j, h, g0):
                self.j = j
                self.h = h
                self.nsk = (j + 1) * DIAG
                self.gs = []
                rem = self.nsk
                g = g0
                while rem > 0:
                    take = min(2, rem)
                    self.gs.append((g, take))
                    rem -= take
                    g += 1
                self.g_end = g
                self.dw = 0      # initialized width (slots) of denw
                self.denw = None
                self.at_ps = None
                self.rbc = None

        def diag_o(p, si):
            """Leading q-columns of diagonal block si that are fully
            masked; score/exp/mask/den/AV all skip them.  Chunk 0 keeps
            full width (cheap, and keeps the very first groups simple)."""
            if p.j == 0:
                return 0
            return max(0, si * 128 - p.j * NS)

        def score_group(p, gidx, gi, si0, G):
            """score MMs + exp for one group; ACT starts ASAP."""
            pool = scwA_pool if gidx % 2 == 0 else scwB_pool
            tag = "scwA" if gidx % 2 == 0 else "scwB"
            scw = pool.tile([128, G * NS], F32, tag=tag,
                            name=f"sc_{p.j}_{p.h}_{gi}")
            os_ = []
            for lg in range(G):
                si = si0 + lg
                o = diag_o(p, si)
                os_.append(o)
                nc.tensor.matmul(
                    scw[:, lg * NS + o:(lg + 1) * NS],
                    krt[:, si * 128:(si + 1) * 128],
                    rts[p.h][:, p.j * NS + o:(p.j + 1) * NS],
                    start=True, stop=True)
            ptw = ptw_pool.tile([128, G * NS], F16, tag="ptw")
            if all(o == 0 for o in os_):
                nc.scalar.activation(ptw[:], scw[:],
                                     mybir.ActivationFunctionType.Exp,
                                     scale=scale)
            else:
                # exp only the written subranges (reading the skipped
                # bytes would alias the pool's previous tile)
                for lg in range(G):
                    o = os_[lg]
                    nc.scalar.activation(
                        ptw[:, lg * NS + o:(lg + 1) * NS],
                        scw[:, lg * NS + o:(lg + 1) * NS],
                        mybir.ActivationFunctionType.Exp, scale=scale)
            return ptw

        def tail_group(p, gi, si0, G, ptw):
            """mask + den-accumulate + AV for one group (lag 2).

            All reads restricted to the columns the score/exp stage
            wrote (diag_o skip); the skipped columns are fully masked
            and contribute nothing."""
            os_ = [diag_o(p, si0 + lg) for lg in range(G)]
            for lg in range(G):
                si = si0 + lg
                o = si * 128 - p.j * NS
                if o >= 0:  # diagonal block: causal mask
                    sk = os_[lg]
                    nc.vector.tensor_mul(
                        ptw[:, lg * NS + sk:(lg + 1) * NS],
                        ptw[:, lg * NS + sk:(lg + 1) * NS],
                        msk[:, (NS - 128) - o + sk:(2 * NS - 128) - o])
            if gi == 0:
                p.denw = den_pool.tile([128, 2 * NS], F16, tag="den",
                                       name=f"den_{p.j}_{p.h}")
                nc.vector.tensor_copy(p.denw[:, 0:G * NS], ptw[:])
                p.dw = G
            elif any(o > 0 for o in os_):
                for lg in range(min(G, p.dw)):
                    sk = os_[lg]
                    nc.vector.tensor_add(
                        p.denw[:, lg * NS + sk:(lg + 1) * NS],
                        p.denw[:, lg * NS + sk:(lg + 1) * NS],
                        ptw[:, lg * NS + sk:(lg + 1) * NS])
            else:
                ga = min(G, p.dw)
                nc.vector.tensor_add(p.denw[:, 0:ga * NS],
                                     p.denw[:, 0:ga * NS],
                                     ptw[:, 0:ga * NS])
                if G > p.dw:
                    nc.vector.tensor_copy(p.denw[:, p.dw * NS:G * NS],
                                          ptw[:, ga * NS:G * NS])
                    p.dw = G
            if gi == 0:
                p.at_ps = atp_pool.tile([128, NS], F32, tag="atp",
                                        name=f"at_{p.j}_{p.h}")
            for lg in range(G):
                si = si0 + lg
                sk = os_[lg]
                nc.tensor.matmul(
                    p.at_ps[:, sk:NS],
                    vnat[:, si * 128:(si + 1) * 128],
                    ptw[:, lg * NS + sk:(lg + 1) * NS],
                    start=(si == 0), stop=(si == p.nsk - 1))

        def fin_a(p):
            """den fold + partition-sum + reciprocal + broadcast."""
            if p.dw >= 2:
                nc.vector.tensor_add(p.denw[:, 0:NS], p.denw[:, 0:NS],
                                     p.denw[:, NS:2 * NS])
            dps = pjd_pool.tile([1, NS], F32, tag="pjd",
                                name=f"dps_{p.j}_{p.h}")
            nc.tensor.matmul(dps[:], ones[:, 0:1], p.denw[:, 0:NS],
                             start=True, stop=True)
            rc = rc_pool.tile([1, NS], F32, tag="rc")
            nc.vector.reciprocal_approx_fast(out=rc[:], in_=dps[:])
            rbc = rbc_pool.tile([128, NS], F32, tag="rbc")
            nc.gpsimd.partition_broadcast(rbc[:], rc[:])
            p.rbc = rbc

        HW_ = HQL * SW
        raws = {}

        def emit_import_dma(m, engine, gate_src=None):
            """Import chunk m's AllToAll result (contiguous 512B runs
            into aot_raw)."""
            aot_raw = aotr_pool.tile([128, N_CORES * HW_], BF16,
                                     tag="aotr", name=f"aotr_{m}")
            if gate_src is not None:
                # a 1-element gpsimd copy the import WAW-depends on.
                # Its SOURCE is data produced late in attention, so the
                # scheduler cannot hoist it (a dep-free memset gets
                # hoisted, and the import's collective-wait then
                # head-of-line-blocks the ring mid-attention).
                nc.gpsimd.tensor_copy(aot_raw[0:1, 0:1],
                                      gate_src[0:1, 0:1])
            src = a2a_out[m].ap().rearrange("(d e f) w -> e d f w",
                                            e=128, f=HQL)
            engine.dma_start(
                aot_raw[:].rearrange("p (d f w) -> p d f w",
                                     d=N_CORES, f=HQL), src)
            raws[m] = aot_raw

        def emit_permute(m):
            """DVE column-permute into aot with cols (q, f, b, w) so
            each head's lhsT is a contiguous [128, NB*SW] slice (matmul
            APs allow only one free dim)."""
            aot_raw = raws[m]
            aot = aot_pool.tile([128, N_CORES * HW_], BF16,
                                tag="aot", name=f"aot_{m}")
            for b in range(NB):
                nc.vector.tensor_copy(
                    aot[:].rearrange("p (q f b w) -> p q f b w",
                                     q=TP, f=HQL, b=NB)[:, :, :, b, :],
                    aot_raw[:, b * TP * HW_:(b + 1) * TP * HW_]
                    .rearrange("p (q f w) -> p q f w", q=TP, f=HQL))
            return aot

        def fin_b(p):
            """normalize + export chunk strips + (maybe) AllToAll.

            Block row layout is (e, local_head) -- e-major -- so the
            phase-3 import gathers contiguous HQL*SW*2-byte runs per
            (partition, src core) instead of SW*2-byte ones."""
            asb = attn_pool.tile([128, NS], BF16, tag="attn")
            nc.vector.tensor_mul(asb[:], p.at_ps[:], p.rbc[:])
            dst = a2a_in[p.j].ap().rearrange("(d e f) w -> e d f w",
                                             e=128, f=HQL)
            nc.sync.dma_start(
                dst[:, :, p.h, :],
                asb[:].rearrange("p (d w) -> p d w", d=NSW))
            if p.h == HQL - 1:
                nc.gpsimd.collective_compute(
                    "AllToAll", mybir.AluOpType.bypass,
                    ins=[a2a_in[p.j].ap().opt()],
                    outs=[a2a_out[p.j].ap().opt()],
                    replica_groups=groups)
                if p.j == NC - 2:
                    # chunk 0's AllToAll completed long ago: pull its
                    # import DMA in now (gated on this pass's rbc, i.e.
                    # real data) so out-proj m=0 has its operand
                    # resident the moment attention ends.  The permute
                    # stays in phase 3 -- emitted here it would sit in
                    # the DVE FIFO waiting on the transfer and stall
                    # the remaining attention tails behind it.
                    emit_import_dma(0, nc.sync, gate_src=p.rbc)

        # ---- merged pipeline ----
        # prelude: chunk-0 projections, then for each chunk j emit its
        # attention groups with chunk j+1's projection units spread
        # between them as PE filler.
        for u in proj_units(0):
            u()

        pend_tails = []          # deque, max depth 2 (lag-2)
        fins = []   # list of [pass, next_stage] with stage in ("a","b")

        def pop_tail():
            tp, tgi, tsi0, tG, tptw = pend_tails.pop(0)
            tail_group(tp, tgi, tsi0, tG, tptw)
            if tgi == len(tp.gs) - 1:
                fins.append([tp, "a"])

        def advance_fins():
            adv = 0
            while fins and (adv == 0 or (len(fins) > 1 and adv < 3)):
                fp, stage = fins[0]
                if stage == "a":
                    fin_a(fp)
                    fins[0][1] = "b"
                else:
                    fin_b(fp)
                    fins.pop(0)
                adv += 1

        g_global = 0
        for j in range(NC):
            items = []
            for h in range(HQL):
                p = Pass(j, h, g_global)
                g_global = p.g_end
                si0 = 0
                for gi, (gidx, G) in enumerate(p.gs):
                    items.append((p, gidx, gi, si0, G))
                    si0 += G
            units = proj_units(j + 1) if j + 1 < NC else []
            cadence = max(1, (len(items) + len(units) - 1)
                          // max(1, len(units)))
            for idx, it in enumerate(items):
                p, gidx, gi, si0, G = it
                ptw = score_group(p, gidx, gi, si0, G)
                if len(pend_tails) >= 2:
                    pop_tail()
                pend_tails.append((p, gi, si0, G, ptw))
                advance_fins()
                if units and (idx + 1) % cadence == 0:
                    units.pop(0)()
            while units:
                units.pop(0)()
        while pend_tails:
            pop_tail()
        for fp, stage in fins:
            if stage == "a":
                fin_a(fp)
            fin_b(fp)

        # ---- phase 3: output projection (all after last AllToAll
        # trigger; chunk m's imports wait only on AllToAll #m, so PE
        # chews chunks 0-2 while #3 is in flight) ----
        NO = D // NS
        for m in range(NC):
            if m not in raws:
                # remaining imports go on the SCALAR DGE ring, pinned
                # past the end of the model schedule: on the sync ring
                # their collective-wait head-of-line-blocks the exports
                # / final out-writes queued behind them, and if hoisted
                # earlier on the scalar ring they'd block the exp ACTs.
                # Scalar is idle once attention ends, which is exactly
                # when these need to issue.
                with tc.tile_wait_until(1.0):
                    emit_import_dma(m, nc.scalar)
            aot = emit_permute(m)
            for n in range(NO):
                pso = atp_pool.tile([P2, NS], F32, tag="atp",
                                    name=f"pso_{m}_{n}")
                for ht in range(NH):
                    nc.tensor.matmul(
                        pso[:], aot[:, ht * P2:(ht + 1) * P2],
                        wo_t(ht)[:, n * NS:(n + 1) * NS],
                        start=(ht == 0), stop=(ht == NH - 1))
                ob = osb_pool.tile([P2, NS], F32, tag="osb")
                nc.vector.tensor_copy(ob[:], pso[:])
                for beta in range(NB):
                    r0 = beta * OW + m * SW
                    nc.sync.dma_start(
                        out_e[r0:r0 + SW, n * NS:(n + 1) * NS],
                        ob[beta * SW:(beta + 1) * SW, :])

    nc.compile()
    return nc


def host_prepare(x, wq, wk, wv, wo, S, D, HQL, NS):
    """Layout-only host prep: slice/transpose/cast + rope tables + mask."""
    hd = HD
    MQ = HQL * hd
    bf = ml_dtypes.bfloat16

    perm = np.concatenate([np.arange(0, hd, 2), np.arange(1, hd, 2)])

    def permute_heads(w):
        nh = w.shape[0] // hd
        w = w.reshape(nh, hd, -1)[:, perm, :]
        return w.reshape(nh * hd, -1)

    wq_p = permute_heads(wq)
    wk_p = permute_heads(wk)

    inv_freq = 1.0 / (ROPE_THETA ** (np.arange(0, hd, 2, dtype=np.float64)
                                     / hd))
    ang = np.arange(S, dtype=np.float64)[None, :] * inv_freq[:, None]
    cc = np.cos(ang)
    ss = np.sin(ang)
    # rope tables over full 128 partitions: rows 0:64 = even dims,
    # rows 64:128 = odd dims.  out = t*ccx + swap(t)*ssx.
    ccx = np.concatenate([cc, cc], axis=0).astype(bf)
    ssx = np.concatenate([-ss, ss], axis=0).astype(bf)

    pswap = np.zeros((128, 128), dtype=bf)
    pswap[np.arange(64), np.arange(64, 128)] = 1.0
    pswap[np.arange(64, 128), np.arange(64)] = 1.0

    p = np.arange(128)[:, None]
    c = np.arange(NS + 384)[None, :]
    mski = (p <= c - (NS - 128)).astype(np.float16)

    woT = np.ascontiguousarray(wo.T).astype(bf)

    in_maps = []
    for core in range(N_CORES):
        b = core // TP
        r = core % TP
        qsl = slice(r * MQ, (r + 1) * MQ)
        ksl = slice(r * hd, (r + 1) * hd)
        in_maps.append({
            "xT": np.ascontiguousarray(x[b].T).astype(bf),
            "wqT": np.ascontiguousarray(wq_p[qsl].T).astype(bf),
            "wkT": np.ascontiguousarray(wk_p[ksl].T).astype(bf),
            "wvT": np.ascontiguousarray(wv[ksl].T).astype(bf),
            "woT": woT,
            "ccx": ccx, "ssx": ssx, "pswap": pswap, "mask": mski,
        })
    return in_maps


_NC_CACHE = {}


def get_graph(S=2048, D=2048, HQL=4, NS=512):
    key = (S, D, HQL, NS)
    if key not in _NC_CACHE:
        _NC_CACHE[key] = build_graph(S, D, HQL, NS)
    return _NC_CACHE[key]


def unshard_out(results, B, S, D, NS):
    """results[core]["out"] is [B*NC*SW, D]; core c's rows (beta, chunk
    m, w) hold output cols m*NS + c*SW + w of batch beta."""
    out = np.empty((B, S, D), dtype=np.float32)
    NC = S // NS
    SW = NS // N_CORES
    OW = NC * SW
    for core in range(N_CORES):
        r = results[core]["out"]
        for beta in range(B):
            for m in range(NC):
                c0 = m * NS + core * SW
                r0 = beta * OW + m * SW
                out[beta, c0:c0 + SW, :] = r[r0:r0 + SW, :]
    return out


def kernel(x, wq, wk, wv, wo, trace=False):
    B, S, D = x.shape
    HQL = (wq.shape[0] // HD) // TP
    NS = 512
    nc = get_graph(S, D, HQL, NS)
    in_maps = host_prepare(x, wq, wk, wv, wo, S, D, HQL, NS)
    res = run_bass_kernel_spmd(nc, in_maps, core_ids=list(range(N_CORES)),
                               trace=trace)
    out = unshard_out(res.results, B, S, D, NS)
    if trace:
        kernel.last_exec_time_ns = res.exec_time_ns
        kernel.last_results = res
    return out


# revision 40
# speedup vs baseline: 1.3744x; 1.3744x over previous
"""Trainium2 Bass kernel for GQA causal attention (B=2, S=2048, D=2048,
16 q-heads / 4 kv-heads, head_dim=128, interleaved RoPE).

Sharding: DP=2 over batch x TP=4 over head groups (8 cores).
Core c: batch b=c//4, rank r=c%4 -> q-heads [4r,4r+4), kv-head r.
Each core computes its heads' attention output (transposed layout [e,s]);
after each 512-col chunk j an AllToAll reshards that chunk's columns to
their owner cores (core c owns cols j*512 + c*64 of both batches); the
output projection for all four chunks runs after the last trigger so the
final AllToAll hides under chunk 0-2 out-proj work.

v7: single merged projection/attention pipeline (trace-driven).
 - chunk j+1's projection m-tiles (k, v, q0..q3, all pure-PE work) are
   interleaved between chunk j's attention groups, so every exp ACT /
   mask / softmax-finalize chain has PE cover and the phase-1 -> 2
   boundary stalls disappear; attention chunk 0 likewise covers the
   xT chunk-1 DMA.  PSUM rebalanced for this: two 2-bank score pools
   (uniform 2-tile groups), 2x1-bank AV accumulators, and a 2x1-bank
   shared pool for proj psum / rope swap / v-transpose / den sums.
 - every bulk load is ONE multi-block DMA (a single InstDMACopy is
   split across all 16 SDMA engines): per-128KB-instruction issue cost
   capped the stream at ~200GB/s and starved the PE at start.
 - attention tail (mask+den+AV) lags the score/exp pipeline by TWO
   groups (~2us of PE cover per exp ACT).
 - one AllToAll per chunk (0.5MB); a2a block rows are (e, local_head)
   so the phase-3 import gathers 512B runs (one DMA per chunk) into a
   raw tile, then a DVE column-permute makes each head's lhsT a
   contiguous [128, 128] slice.  Imports issue on the scalar DGE ring
   pinned past the end of the model schedule: on the sync ring their
   collective-wait head-of-line-blocks the exports / out-writes queued
   behind them.
"""

import math
import sys

sys.path.insert(0, "/opt/trn_rl_repo")

from contextlib import ExitStack

import ml_dtypes
import numpy as np

import concourse.bass as bass
import concourse.mybir as mybir
import concourse.tile as tile
from concourse import bacc
from concourse.bass_utils import run_bass_kernel_spmd
from concourse.masks import make_identity

BF16 = mybir.dt.bfloat16
F16 = mybir.dt.float16
F32 = mybir.dt.float32

N_HEADS = 16
N_KV_HEADS = 4
HD = 128
ROPE_THETA = 10000.0
TP = 4
N_CORES = 8


def build_graph(S=2048, D=2048, HQL=4, NS=512):
    """Per-core SPMD graph. HQL = local q heads; local kv heads = 1."""
    hd = HD
    ND = D // 128          # d-tiles (projection contraction tiles)
    NC = S // NS           # s-chunks == AllToAll count
    MQ = HQL * hd          # local q width
    DIAG = NS // 128       # sk-tiles per chunk needing a causal mask
    NB = N_CORES // TP     # batches
    SW = NS // N_CORES     # strip width per (chunk, dst core)
    NSW = NS // SW         # dst blocks per export (= N_CORES)
    OW = NC * SW           # out cols per core per batch
    P2 = NB * SW           # fused out-proj psum partition rows
    scale = 1.0 / math.sqrt(hd)
    NH = TP * HQL          # global head count

    nc = bacc.Bacc("TRN2", target_bir_lowering=False, debug=False,
                   num_devices=N_CORES)

    xT_e = nc.dram_tensor("xT", [D, S], BF16, kind="ExternalInput").ap()
    wqT_e = nc.dram_tensor("wqT", [D, MQ], BF16, kind="ExternalInput").ap()
    wkT_e = nc.dram_tensor("wkT", [D, hd], BF16, kind="ExternalInput").ap()
    wvT_e = nc.dram_tensor("wvT", [D, hd], BF16, kind="ExternalInput").ap()
    woT_e = nc.dram_tensor("woT", [NH * hd, D], BF16,
                           kind="ExternalInput").ap()
    ccx_e = nc.dram_tensor("ccx", [128, S], BF16, kind="ExternalInput").ap()
    ssx_e = nc.dram_tensor("ssx", [128, S], BF16, kind="ExternalInput").ap()
    psw_e = nc.dram_tensor("pswap", [128, 128], BF16,
                           kind="ExternalInput").ap()
    mask_e = nc.dram_tensor("mask", [128, NS + 384], F16,
                            kind="ExternalInput").ap()
    out_e = nc.dram_tensor("out", [NB * OW, D], F32,
                           kind="ExternalOutput").ap()

    a2a_in = [nc.dram_tensor(f"a2a_in{m}", [N_CORES * MQ, SW], BF16)
              for m in range(NC)]
    a2a_out = [nc.dram_tensor(f"a2a_out{m}", [N_CORES * MQ, SW], BF16)
               for m in range(NC)]
    groups = [list(range(N_CORES))]

    with tile.TileContext(nc) as tc, ExitStack() as ctx:
        ep = ctx.enter_context
        const_pool = ep(tc.tile_pool(name="const", bufs=1))
        rt_pool = ep(tc.tile_pool(name="rt", bufs=HQL + 1))
        vst_pool = ep(tc.tile_pool(name="vst", bufs=1))
        ptw_pool = ep(tc.tile_pool(name="ptw", bufs=3))
        den_pool = ep(tc.tile_pool(name="den", bufs=2))
        rc_pool = ep(tc.tile_pool(name="rc", bufs=1))
        rbc_pool = ep(tc.tile_pool(name="rbc", bufs=2))
        attn_pool = ep(tc.tile_pool(name="attn", bufs=3))
        osb_pool = ep(tc.tile_pool(name="osb", bufs=2))
        wo_pool = ep(tc.tile_pool(name="wo", bufs=1))
        aot_pool = ep(tc.tile_pool(name="aot", bufs=2))
        aotr_pool = ep(tc.tile_pool(name="aotr", bufs=1))
        xt_pool = ep(tc.tile_pool(name="xt", bufs=2))
        wq_pool = ep(tc.tile_pool(name="wqp", bufs=1))
        wkv_pool = ep(tc.tile_pool(name="wkv", bufs=1))
        stage_pool = ep(tc.tile_pool(name="stg", bufs=3))
        tmp_pool = ep(tc.tile_pool(name="tmp", bufs=4))
        # PSUM: 8 banks, statically partitioned:
        #   scwA 1x2 + scwB 1x2 + atp 2x1 + pjd 2x1 = 8
        # pjd is shared by proj psum / rope-swap / v-transpose / den-sum
        scwA_pool = ep(tc.tile_pool(name="scwA", bufs=1, space="PSUM"))
        scwB_pool = ep(tc.tile_pool(name="scwB", bufs=1, space="PSUM"))
        atp_pool = ep(tc.tile_pool(name="atp", bufs=2, space="PSUM"))
        pjd_pool = ep(tc.tile_pool(name="pjd", bufs=2, space="PSUM"))

        # ---- constants ----
        ident = const_pool.tile([128, 128], BF16, tag="ident")
        make_identity(nc, ident[:])
        ones = const_pool.tile([128, 1], F16, tag="ones")
        nc.gpsimd.memset(ones[:], 1.0)
        warm = const_pool.tile([1, 8], F32, tag="warm")
        nc.gpsimd.memset(warm[:], 0.0)
        # preload the exp ACT table set before attention needs it
        nc.scalar.activation(warm[:], warm[:],
                             mybir.ActivationFunctionType.Exp)
        ccx = const_pool.tile([128, S], BF16, tag="ccx")
        ssx = const_pool.tile([128, S], BF16, tag="ssx")
        pswap = const_pool.tile([128, 128], BF16, tag="pswap")
        msk = const_pool.tile([128, NS + 384], F16, tag="msk")

        vst = vst_pool.tile([128, S], BF16, tag="vst")   # vT staging
        vnat = vst_pool.tile([128, S], BF16, tag="vnat")  # v [sk, e] blocks

        # ---- DMA stream (priority order; each a single multi-block
        # DMA so it spreads across all 16 SDMA engines) ----
        nc.sync.dma_start(pswap[:], psw_e[:])
        wk_all = wkv_pool.tile([128, ND * hd], BF16, tag="wk")
        nc.sync.dma_start(
            wk_all[:].rearrange("p (d h) -> p d h", d=ND),
            wkT_e.rearrange("(d p) h -> p d h", p=128))
        xts = []
        for sp in range(NC):
            xts.append(xt_pool.tile([128, ND * NS], BF16, tag="xt",
                                    name=f"xt{sp}"))
        # chunk 0 in pieces (smallest first) so the k-projection's
        # leading d-tiles can start as early as possible
        if ND >= 8:
            pieces = [2, 2] + [4] * ((ND - 4) // 4)
        else:
            pieces = [2] * (ND // 2)
        q0 = 0
        for pc in pieces:
            nc.sync.dma_start(
                xts[0][:, q0 * NS:(q0 + pc) * NS].rearrange(
                    "p (d s) -> p d s", d=pc),
                xT_e.rearrange("(d p) s -> p d s",
                               p=128)[:, q0:q0 + pc, 0:NS])
            q0 += pc
        wv_all = wkv_pool.tile([128, ND * hd], BF16, tag="wv")
        nc.sync.dma_start(
            wv_all[:].rearrange("p (d h) -> p d h", d=ND),
            wvT_e.rearrange("(d p) h -> p d h", p=128))
        # rope tables: chunk-0 columns first, the rest after xT chunk 1
        nc.sync.dma_start(ccx[:, 0:NS], ccx_e[:, 0:NS])
        nc.sync.dma_start(ssx[:, 0:NS], ssx_e[:, 0:NS])
        # wq per head, in consumption order, so xT chunk 1 isn't stuck
        # behind the full 2MB
        wq_all = wq_pool.tile([128, ND * MQ], BF16, tag="wq")
        for h in range(HQL):
            nc.sync.dma_start(
                wq_all[:].rearrange("p (d m) -> p d m",
                                    d=ND)[:, :, h * hd:(h + 1) * hd],
                wqT_e.rearrange("(d p) m -> p d m",
                                p=128)[:, :, h * hd:(h + 1) * hd])
        nc.sync.dma_start(
            xts[1][:].rearrange("p (d s) -> p d s", d=ND),
            xT_e.rearrange("(d p) s -> p d s", p=128)[:, :, NS:2 * NS])
        nc.sync.dma_start(
            ccx[:, NS:].rearrange("p (b c) -> p b c", b=NC - 1),
            ccx_e[:, NS:].rearrange("p (b c) -> p b c", b=NC - 1))
        nc.sync.dma_start(
            ssx[:, NS:].rearrange("p (b c) -> p b c", b=NC - 1),
            ssx_e[:, NS:].rearrange("p (b c) -> p b c", b=NC - 1))
        nc.sync.dma_start(msk[:], mask_e[:])
        for sp in range(2, NC):
            nc.sync.dma_start(
                xts[sp][:].rearrange("p (d s) -> p d s", d=ND),
                xT_e.rearrange("(d p) s -> p d s",
                               p=128)[:, :, sp * NS:(sp + 1) * NS])
        wo_all = wo_pool.tile([128, NH * D], BF16, tag="wo")
        nc.sync.dma_start(
            wo_all[:].rearrange("p (t c) -> p t c", t=NH),
            woT_e.rearrange("(t p) c -> p t c", p=128))

        def wo_t(ht):
            return wo_all[:, ht * D:(ht + 1) * D]

        # ---- projections + rope (emitted as per-chunk unit lists,
        # interleaved into the previous chunk's attention below) ----
        # Rope is software-pipelined one m-tile behind the projection
        # matmuls: the swap-MM of item k is emitted after the proj
        # group of item k+1 so it never stalls the PE waiting for the
        # ACT psum->stg copy.
        rope_pend = []   # queue of (stg, swap-psum, rt_tile, ssl)

        def rope_tail():
            if not rope_pend:
                return
            stg, ps2, rt_tile, ssl = rope_pend.pop(0)
            t1 = tmp_pool.tile([128, NS], BF16, tag="tmp")
            nc.vector.tensor_mul(t1[:], stg[:], ccx[:, ssl])
            t2 = tmp_pool.tile([128, NS], BF16, tag="tmp")
            nc.vector.tensor_mul(t2[:], ps2[:], ssx[:, ssl])
            nc.vector.tensor_add(rt_tile[:, ssl], t1[:], t2[:])

        def rope_swap_mm():
            if not rope_pend:
                return
            stg, _, rt_tile, ssl = rope_pend[0]
            ps2 = pjd_pool.tile([128, NS], F32, tag="pjd",
                                name="ps_swap")
            nc.tensor.matmul(ps2[:], pswap[:], stg[:],
                             start=True, stop=True)
            rope_pend[0] = (stg, ps2, rt_tile, ssl)

        def proj_one(lhs_all, lw, mslice, is_v, rt_tile, sp):
            ssl = slice(sp * NS, (sp + 1) * NS)
            ps = pjd_pool.tile([128, NS], F32, tag="pjd", name="psp")
            for d in range(ND):
                nc.tensor.matmul(
                    ps[:],
                    lhs_all[:, d * lw + mslice.start:
                            d * lw + mslice.stop],
                    xts[sp][:, d * NS:(d + 1) * NS],
                    start=(d == 0), stop=(d == ND - 1))
            rope_swap_mm()
            rope_tail()
            if is_v:
                nc.scalar.copy(vst[:, ssl], ps[:])
            else:
                stg = stage_pool.tile([128, NS], BF16, tag="stg")
                nc.scalar.copy(stg[:], ps[:])
                rope_pend.append((stg, None, rt_tile, ssl))

        def v_transpose(sp):
            for lt in range(DIAG):
                st = sp * DIAG + lt
                tpp = pjd_pool.tile([128, 128], BF16, tag="pjd",
                                    name="pst")
                nc.tensor.transpose(
                    tpp[:], vst[:, st * 128:(st + 1) * 128], ident[:])
                nc.scalar.copy(vnat[:, st * 128:(st + 1) * 128], tpp[:])

        def rope_flush():
            rope_swap_mm()
            rope_tail()
            rope_swap_mm()
            rope_tail()

        krt = rt_pool.tile([128, S], BF16, tag="rt", name="rtk")
        rts = [rt_pool.tile([128, S], BF16, tag="rt", name=f"rtq{h}")
               for h in range(HQL)]

        def proj_units(sp):
            """Pure-PE unit closures projecting chunk sp."""
            us = [lambda sp=sp: proj_one(wk_all, hd, slice(0, hd),
                                         False, krt, sp),
                  lambda sp=sp: proj_one(wv_all, hd, slice(0, hd),
                                         True, None, sp),
                  lambda sp=sp: v_transpose(sp)]
            for h in range(HQL):
                us.append(lambda sp=sp, h=h: proj_one(
                    wq_all, MQ, slice(h * hd, (h + 1) * hd),
                    False, rts[h], sp))
            us.append(rope_flush)
            return us

        # ---- attention machinery ----
        # One head per pass; sk-tiles processed in uniform 2-tile
        # groups alternating between the two 2-bank score pools, ONE
        # exp ACTIVATE per group.  The mask/den/AV tail lags the
        # score+exp pipeline by TWO groups; a pass's finalize (den
        # fold+reduce, recip, broadcast) and export (normalize + DMA +
        # AllToAll) are deferred into the following groups.
        class Pass:
            def __init__(self, j, h, g0):
                self.j = j
                self.h = h
                self.nsk = (j + 1) * DIAG
                self.gs = []
                rem = self.nsk
                g = g0
                while rem > 0:
                    take = min(2, rem)
                    self.gs.append((g, take))
                    rem -= take
                    g += 1
                self.g_end = g
                self.dw = 0      # initialized width (slots) of denw
                self.denw = None
                self.at_ps = None
                self.rbc = None

        def diag_o(p, si):
            """Leading q-columns of diagonal block si that are fully
            masked; score/exp/mask/den/AV all skip them.  Chunk 0 keeps
            full width (cheap, and keeps the very first groups simple)."""
            if p.j == 0:
                return 0
            return max(0, si * 128 - p.j * NS)

        def score_group(p, gidx, gi, si0, G):
            """score MMs + exp for one group; ACT starts ASAP."""
            pool = scwA_pool if gidx % 2 == 0 else scwB_pool
            tag = "scwA" if gidx % 2 == 0 else "scwB"
            scw = pool.tile([128, G * NS], F32, tag=tag,
                            name=f"sc_{p.j}_{p.h}_{gi}")
            os_ = []
            for lg in range(G):
                si = si0 + lg
                o = diag_o(p, si)
                os_.append(o)
                nc.tensor.matmul(
                    scw[:, lg * NS + o:(lg + 1) * NS],
                    krt[:, si * 128:(si + 1) * 128],
                    rts[p.h][:, p.j * NS + o:(p.j + 1) * NS],
                    start=True, stop=True)
            ptw = ptw_pool.tile([128, G * NS], F16, tag="ptw")
            if all(o == 0 for o in os_):
                nc.scalar.activation(ptw[:], scw[:],
                                     mybir.ActivationFunctionType.Exp,
                                     scale=scale)
            else:
                # exp only the written subranges (reading the skipped
                # bytes would alias the pool's previous tile)
                for lg in range(G):
                    o = os_[lg]
                    nc.scalar.activation(
                        ptw[:, lg * NS + o:(lg + 1) * NS],
                        scw[:, lg * NS + o:(lg + 1) * NS],
                        mybir.ActivationFunctionType.Exp, scale=scale)
            return ptw

        def tail_group(p, gi, si0, G, ptw):
            """mask + den-accumulate + AV for one group (lag 2).

            All reads restricted to the columns the score/exp stage
            wrote (diag_o skip); the skipped columns are fully masked
            and contribute nothing."""
            os_ = [diag_o(p, si0 + lg) for lg in range(G)]
            for lg in range(G):
                si = si0 + lg
                o = si * 128 - p.j * NS
                if o >= 0:  # diagonal block: causal mask
                    sk = os_[lg]
                    nc.vector.tensor_mul(
                        ptw[:, lg * NS + sk:(lg + 1) * NS],
                        ptw[:, lg * NS + sk:(lg + 1) * NS],
                        msk[:, (NS - 128) - o + sk:(2 * NS - 128) - o])
            if gi == 0:
                p.denw = den_pool.tile([128, 2 * NS], F16, tag="den",
                                       name=f"den_{p.j}_{p.h}")
                nc.vector.tensor_copy(p.denw[:, 0:G * NS], ptw[:])
                p.dw = G
            elif any(o > 0 for o in os_):
                for lg in range(min(G, p.dw)):
                    sk = os_[lg]
                    nc.vector.tensor_add(
                        p.denw[:, lg * NS + sk:(lg + 1) * NS],
                        p.denw[:, lg * NS + sk:(lg + 1) * NS],
                        ptw[:, lg * NS + sk:(lg + 1) * NS])
            else:
                ga = min(G, p.dw)
                nc.vector.tensor_add(p.denw[:, 0:ga * NS],
                                     p.denw[:, 0:ga * NS],
                                     ptw[:, 0:ga * NS])
                if G > p.dw:
                    nc.vector.tensor_copy(p.denw[:, p.dw * NS:G * NS],
                                          ptw[:, ga * NS:G * NS])
                    p.dw = G
            if gi == 0:
                p.at_ps = atp_pool.tile([128, NS], F32, tag="atp",
                                        name=f"at_{p.j}_{p.h}")
            for lg in range(G):
                si = si0 + lg
                sk = os_[lg]
                nc.tensor.matmul(
                    p.at_ps[:, sk:NS],
                    vnat[:, si * 128:(si + 1) * 128],
                    ptw[:, lg * NS + sk:(lg + 1) * NS],
                    start=(si == 0), stop=(si == p.nsk - 1))

        def fin_a(p):
            """den fold + partition-sum + reciprocal + broadcast."""
            if p.dw >= 2:
                nc.vector.tensor_add(p.denw[:, 0:NS], p.denw[:, 0:NS],
                                     p.denw[:, NS:2 * NS])
            dps = pjd_pool.tile([1, NS], F32, tag="pjd",
                                name=f"dps_{p.j}_{p.h}")
            nc.tensor.matmul(dps[:], ones[:, 0:1], p.denw[:, 0:NS],
                             start=True, stop=True)
            rc = rc_pool.tile([1, NS], F32, tag="rc")
            nc.vector.reciprocal_approx_fast(out=rc[:], in_=dps[:])
            rbc = rbc_pool.tile([128, NS], F32, tag="rbc")
            nc.gpsimd.partition_broadcast(rbc[:], rc[:])
            p.rbc = rbc

        HW_ = HQL * SW
        raws = {}

        def emit_import_dma(m, engine, gate_src=None):
            """Import chunk m's AllToAll result (contiguous 512B runs
            into aot_raw)."""
            aot_raw = aotr_pool.tile([128, N_CORES * HW_], BF16,
                                     tag="aotr", name=f"aotr_{m}")
            if gate_src is not None:
                # a 1-element gpsimd copy the import WAW-depends on.
                # Its SOURCE is data produced late in attention, so the
                # scheduler cannot hoist it (a dep-free memset gets
                # hoisted, and the import's collective-wait then
                # head-of-line-blocks the ring mid-attention).
                nc.gpsimd.tensor_copy(aot_raw[0:1, 0:1],
                                      gate_src[0:1, 0:1])
            src = a2a_out[m].ap().rearrange("(d e f) w -> e d f w",
                                            e=128, f=HQL)
            engine.dma_start(
                aot_raw[:].rearrange("p (d f w) -> p d f w",
                                     d=N_CORES, f=HQL), src)
            raws[m] = aot_raw

        def emit_permute(m):
            """DVE column-permute into aot with cols (q, f, b, w) so
            each head's lhsT is a contiguous [128, NB*SW] slice (matmul
            APs allow only one free dim)."""
            aot_raw = raws[m]
            aot = aot_pool.tile([128, N_CORES * HW_], BF16,
                                tag="aot", name=f"aot_{m}")
            for b in range(NB):
                nc.vector.tensor_copy(
                    aot[:].rearrange("p (q f b w) -> p q f b w",
                                     q=TP, f=HQL, b=NB)[:, :, :, b, :],
                    aot_raw[:, b * TP * HW_:(b + 1) * TP * HW_]
                    .rearrange("p (q f w) -> p q f w", q=TP, f=HQL))
            return aot

        def fin_b(p):
            """normalize + export chunk strips + (maybe) AllToAll.

            Block row layout is (e, local_head) -- e-major -- so the
            phase-3 import gathers contiguous HQL*SW*2-byte runs per
            (partition, src core) instead of SW*2-byte ones."""
            asb = attn_pool.tile([128, NS], BF16, tag="attn")
            nc.vector.tensor_mul(asb[:], p.at_ps[:], p.rbc[:])
            dst = a2a_in[p.j].ap().rearrange("(d e f) w -> e d f w",
                                             e=128, f=HQL)
            nc.sync.dma_start(
                dst[:, :, p.h, :],
                asb[:].rearrange("p (d w) -> p d w", d=NSW))
            if p.h == HQL - 1:
                nc.gpsimd.collective_compute(
                    "AllToAll", mybir.AluOpType.bypass,
                    ins=[a2a_in[p.j].ap().opt()],
                    outs=[a2a_out[p.j].ap().opt()],
                    replica_groups=groups)
                if p.j == NC - 2:
                    # chunk 0's AllToAll completed long ago: pull its
                    # import DMA in now (gated on this pass's rbc, i.e.
                    # real data) so out-proj m=0 has its operand
                    # resident the moment attention ends.  The permute
                    # stays in phase 3 -- emitted here it would sit in
                    # the DVE FIFO waiting on the transfer and stall
                    # the remaining attention tails behind it.
                    emit_import_dma(0, nc.sync, gate_src=p.rbc)

        # ---- merged pipeline ----
        # prelude: chunk-0 projections, then for each chunk j emit its
        # attention groups with chunk j+1's projection units spread
        # between them as PE filler.
        for u in proj_units(0):
            u()

        pend_tails = []          # deque, max depth 2 (lag-2)
        fins = []   # list of [pass, next_stage] with stage in ("a","b")

        def pop_tail():
            tp, tgi, tsi0, tG, tptw = pend_tails.pop(0)
            tail_group(tp, tgi, tsi0, tG, tptw)
            if tgi == len(tp.gs) - 1:
                fins.append([tp, "a"])

        def advance_fins():
            adv = 0
            while fins and (adv == 0 or (len(fins) > 1 and adv < 3)):
                fp, stage = fins[0]
                if stage == "a":
                    fin_a(fp)
                    fins[0][1] = "b"
                else:
                    fin_b(fp)
                    fins.pop(0)
                adv += 1

        g_global = 0
        for j in range(NC):
            items = []
            for h in range(HQL):
                p = Pass(j, h, g_global)
                g_global = p.g_end
                si0 = 0
                for gi, (gidx, G) in enumerate(p.gs):
                    items.append((p, gidx, gi, si0, G))
                    si0 += G
            units = proj_units(j + 1) if j + 1 < NC else []
            cadence = max(1, (len(items) + len(units) - 1)
                          // max(1, len(units)))
            for idx, it in enumerate(items):
                p, gidx, gi, si0, G = it
                ptw = score_group(p, gidx, gi, si0, G)
                if len(pend_tails) >= 2:
                    pop_tail()
                pend_tails.append((p, gi, si0, G, ptw))
                advance_fins()
                if units and (idx + 1) % cadence == 0:
                    units.pop(0)()
            while units:
                units.pop(0)()
        while pend_tails:
            pop_tail()
        for fp, stage in fins:
            if stage == "a":
                fin_a(fp)
            fin_b(fp)

        # ---- phase 3: output projection (all after last AllToAll
        # trigger; chunk m's imports wait only on AllToAll #m, so PE
        # chews chunks 0-2 while #3 is in flight) ----
        NO = D // NS
        for m in range(NC):
            if m not in raws:
                # remaining imports go on the SCALAR DGE ring, pinned
                # past the end of the model schedule: on the sync ring
                # their collective-wait head-of-line-blocks the exports
                # / final out-writes queued behind them, and if hoisted
                # earlier on the scalar ring they'd block the exp ACTs.
                # Scalar is idle once attention ends, which is exactly
                # when these need to issue.
                with tc.tile_wait_until(1.0):
                    emit_import_dma(m, nc.scalar)
            aot = emit_permute(m)
            for n in range(NO):
                pso = atp_pool.tile([P2, NS], F32, tag="atp",
                                    name=f"pso_{m}_{n}")
                for ht in range(NH):
                    nc.tensor.matmul(
                        pso[:], aot[:, ht * P2:(ht + 1) * P2],
                        wo_t(ht)[:, n * NS:(n + 1) * NS],
                        start=(ht == 0), stop=(ht == NH - 1))
                ob = osb_pool.tile([P2, NS], F32, tag="osb")
                nc.vector.tensor_copy(ob[:], pso[:])
                for beta in range(NB):
                    r0 = beta * OW + m * SW
                    nc.sync.dma_start(
                        out_e[r0:r0 + SW, n * NS:(n + 1) * NS],
                        ob[beta * SW:(beta + 1) * SW, :])

    nc.compile()
    return nc


def host_prepare(x, wq, wk, wv, wo, S, D, HQL, NS):
    """Layout-only host prep: slice/transpose/cast + rope tables + mask."""
    hd = HD
    MQ = HQL * hd
    bf = ml_dtypes.bfloat16

    perm = np.concatenate([np.arange(0, hd, 2), np.arange(1, hd, 2)])

    def permute_heads(w):
        nh = w.shape[0] // hd
        w = w.reshape(nh, hd, -1)[:, perm, :]
        return w.reshape(nh * hd, -1)

    wq_p = permute_heads(wq)
    wk_p = permute_heads(wk)

    inv_freq = 1.0 / (ROPE_THETA ** (np.arange(0, hd, 2, dtype=np.float64)
                                     / hd))
    ang = np.arange(S, dtype=np.float64)[None, :] * inv_freq[:, None]
    cc = np.cos(ang)
    ss = np.sin(ang)
    # rope tables over full 128 partitions: rows 0:64 = even dims,
    # rows 64:128 = odd dims.  out = t*ccx + swap(t)*ssx.
    ccx = np.concatenate([cc, cc], axis=0).astype(bf)
    ssx = np.concatenate([-ss, ss], axis=0).astype(bf)

    pswap = np.zeros((128, 128), dtype=bf)
    pswap[np.arange(64), np.arange(64, 128)] = 1.0
    pswap[np.arange(64, 128), np.arange(64)] = 1.0

    p = np.arange(128)[:, None]
    c = np.arange(NS + 384)[None, :]
    mski = (p <= c - (NS - 128)).astype(np.float16)

    woT = np.ascontiguousarray(wo.T).astype(bf)

    in_maps = []
    for core in range(N_CORES):
        b = core // TP
        r = core % TP
        qsl = slice(r * MQ, (r + 1) * MQ)
        ksl = slice(r * hd, (r + 1) * hd)
        in_maps.append({
            "xT": np.ascontiguousarray(x[b].T).astype(bf),
            "wqT": np.ascontiguousarray(wq_p[qsl].T).astype(bf),
            "wkT": np.ascontiguousarray(wk_p[ksl].T).astype(bf),
            "wvT": np.ascontiguousarray(wv[ksl].T).astype(bf),
            "woT": woT,
            "ccx": ccx, "ssx": ssx, "pswap": pswap, "mask": mski,
        })
    return in_maps


_NC_CACHE = {}


def get_graph(S=2048, D=2048, HQL=4, NS=512):
    key = (S, D, HQL, NS)
    if key not in _NC_CACHE:
        _NC_CACHE[key] = build_graph(S, D, HQL, NS)
    return _NC_CACHE[key]


def unshard_out(results, B, S, D, NS):
    """results[core]["out"] is [B*NC*SW, D]; core c's rows (beta, chunk
    m, w) hold output cols m*NS + c*SW + w of batch beta."""
    out = np.empty((B, S, D), dtype=np.float32)
    NC = S // NS
    SW = NS // N_CORES
    OW = NC * SW
    for core in range(N_CORES):
        r = results[core]["out"]
        for beta in range(B):
            for m in range(NC):
                c0 = m * NS + core * SW
                r0 = beta * OW + m * SW
                out[beta, c0:c0 + SW, :] = r[r0:r0 + SW, :]
    return out


def kernel(x, wq, wk, wv, wo, trace=False):
    B, S, D = x.shape
    HQL = (wq.shape[0] // HD) // TP
    NS = 512
    nc = get_graph(S, D, HQL, NS)
    in_maps = host_prepare(x, wq, wk, wv, wo, S, D, HQL, NS)
    res = run_bass_kernel_spmd(nc, in_maps, core_ids=list(range(N_CORES)),
                               trace=trace)
    out = unshard_out(res.results, B, S, D, NS)
    if trace:
        kernel.last_exec_time_ns = res.exec_time_ns
        kernel.last_results = res
    return out
